# revision 1
# baseline (speedup 1.0000x reference)
"""Position-attention layer (dense_transformer) for Trainium2, 8 NeuronCores.

Data-parallel over batch B=8: one batch element per core. Per core:
  q = relu((sq*Wq) @ x + bq)      [80, 4096]   (scales folded into weights on host)
  k = relu((sk*Wk) @ x + bk)      [80, 4096]
  vT = relu(x^T @ (sv*Wv)^T + bv) [4096, 512]  (computed directly transposed)
  S^T[j,i] = sum_c k[c,j] q[c,i]  (energy, computed transposed, j on partitions)
  P = exp(S^T)                    (no max subtraction: S range is ~[0, 12])
  l[i] = sum_j P[j,i]             (ones-stationary matmuls)
  num[i,c] = sum_j P[j,i] vT[j,c] (PV matmul, i on partitions)
  osc = num / l                   (per-partition scale on eviction)
  out[c,i] = gamma[c,i] * osc^T[c,i] + x[c,i]   (PE transpose + DVE)

Projections + energy run in float32r (full-rate fp32 variant, ~1.5e-4 rel
err); the P/V attention chain runs in bf16 (errors in P largely cancel
between numerator and denominator l, and gamma*out is small next to x).
"""

import sys

sys.path.insert(0, "/opt/trn_rl_repo")

import numpy as np

B, C, H, W = 8, 512, 64, 64
HW = H * W          # 4096
CQK = 80
NCORES = 8
IB = 512            # i-block size for the attention stage
NB = HW // IB       # 8 i-blocks
NS = IB // 128      # 4 i-subtiles per block
NJ = HW // 128      # 32 j-tiles

_STATE = {}


def build_program(loop_reps=None):
    """Build the per-core Bass program. If loop_reps is set, wrap the whole
    kernel body in a hardware For_i loop (used for timing benchmarks only)."""
    from contextlib import ExitStack

    import concourse.bass as bass  # noqa: F401
    import concourse.tile as tile
    from concourse import bacc, mybir

    f32 = mybir.dt.float32
    f32r = mybir.dt.float32r
    bf16 = mybir.dt.bfloat16
    Relu = mybir.ActivationFunctionType.Relu
    Exp = mybir.ActivationFunctionType.Exp
    Copy = mybir.ActivationFunctionType.Copy

    nc = bacc.Bacc("TRN2", target_bir_lowering=False, debug=False)
    x = nc.declare_dram_parameter("x", [C, HW], f32, isOutput=False)
    wqT = nc.declare_dram_parameter("wqT", [C, CQK], f32, isOutput=False)
    wkT = nc.declare_dram_parameter("wkT", [C, CQK], f32, isOutput=False)
    wvT = nc.declare_dram_parameter("wvT", [C, C], f32, isOutput=False)
    bq = nc.declare_dram_parameter("bq", [CQK, 1], f32, isOutput=False)
    bk = nc.declare_dram_parameter("bk", [CQK, 1], f32, isOutput=False)
    bv = nc.declare_dram_parameter("bv", [1, C], f32, isOutput=False)
    gamma = nc.declare_dram_parameter("gamma", [C, HW], f32, isOutput=False)
    onesr = nc.declare_dram_parameter("onesr", [1, 128], f32, isOutput=False)
    eye = nc.declare_dram_parameter("eye", [128, 128], mybir.dt.bfloat16, isOutput=False)
    out = nc.declare_dram_parameter("out", [C, HW], f32, isOutput=True)

    lscratch = nc.dram_tensor("lscratch", [NB, IB], f32)

    def body(tc, ctx):
        persist = ctx.enter_context(tc.tile_pool(name="persist", bufs=1))
        wq_sb = persist.tile([128, 4, CQK], f32r, tag="wq")
        wk_sb = persist.tile([128, 4, CQK], f32r, tag="wk")
        wv_sb = persist.tile([128, 4, C], f32r, tag="wv")
        bq_sb = persist.tile([CQK, 1], f32, tag="bq")
        bk_sb = persist.tile([CQK, 1], f32, tag="bk")
        bv_sb = persist.tile([1, C], f32r, tag="bv")
        onesr_sb = persist.tile([1, 128], f32r, tag="onesr")
        onesc_sb = persist.tile([128, 1], bf16, tag="onesc")
        eye_sb = persist.tile([128, 128], bf16, tag="eye")
        q_sb = persist.tile([CQK, HW], f32r, tag="q")
        k_sb = persist.tile([CQK, HW], f32r, tag="k")
        vT_sb = persist.tile([128, NJ, C], bf16, tag="vT")

        # weight for v first, then x chunk-by-chunk so PE can start early
        nc.sync.dma_start(
            out=wv_sb, in_=wvT[:, :].rearrange("(k p) m -> p k m", p=128).bitcast(f32r)
        )
        nc.sync.dma_start(out=bv_sb, in_=bv[:, :].bitcast(f32r))
        nc.sync.dma_start(out=onesr_sb, in_=onesr[:, :].bitcast(f32r))

        # ---- stage 1: projections ----
        with tc.tile_pool(name="xpool", bufs=1) as xpool:
            x_sb = xpool.tile([128, 4, HW], f32r, tag="x")
            x_re = x[:, :].rearrange("(k p) n -> p k n", p=128).bitcast(f32r)
            for kc in range(4):
                nc.sync.dma_start(out=x_sb[:, kc, :], in_=x_re[:, kc, :])
            nc.sync.dma_start(
                out=wq_sb,
                in_=wqT[:, :].rearrange("(k p) m -> p k m", p=128).bitcast(f32r),
            )
            nc.sync.dma_start(
                out=wk_sb,
                in_=wkT[:, :].rearrange("(k p) m -> p k m", p=128).bitcast(f32r),
            )
            nc.sync.dma_start(out=bq_sb, in_=bq[:, :])
            nc.sync.dma_start(out=bk_sb, in_=bk[:, :])
            nc.vector.memset(onesc_sb, 1.0)
            nc.sync.dma_start(out=eye_sb, in_=eye[:, :])

            # v projection, chunk-outer so MMs start as soon as x chunk 0 lands
            with tc.tile_pool(name="ps1v", bufs=8, space="PSUM") as ps1v:
                for jg in range(NJ // 8):
                    pvs = [
                        ps1v.tile([128, C], f32, tag="pv", name=f"pv{jg}_{jj}")
                        for jj in range(8)
                    ]
                    for kc in range(4):
                        for jj in range(8):
                            j = jg * 8 + jj
                            nc.tensor.matmul(
                                pvs[jj],
                                x_sb[:, kc, j * 128 : (j + 1) * 128],
                                wv_sb[:, kc, :],
                                start=(kc == 0),
                                stop=False,
                            )
                    for jj in range(8):
                        j = jg * 8 + jj
                        nc.tensor.matmul(pvs[jj], onesr_sb, bv_sb, start=False, stop=True)
                        nc.scalar.activation(
                            out=vT_sb[:, j, :], in_=pvs[jj], func=Relu, bias=0.0, scale=1.0
                        )
            # k then q (energy needs all of k but only the current q i-block)
            with tc.tile_pool(name="ps1", bufs=2, space="PSUM") as ps1:
                for w_sb, b_sb, dst in ((wk_sb, bk_sb, k_sb), (wq_sb, bq_sb, q_sb)):
                    for n in range(HW // 512):
                        pq = ps1.tile([CQK, 512], f32, tag="pq")
                        for kc in range(4):
                            nc.tensor.matmul(
                                pq,
                                w_sb[:, kc, :],
                                x_sb[:, kc, n * 512 : (n + 1) * 512],
                                start=(kc == 0),
                                stop=(kc == 3),
                            )
                        nc.scalar.activation(
                            out=dst[:, n * 512 : (n + 1) * 512],
                            in_=pq,
                            func=Relu,
                            bias=b_sb,
                            scale=1.0,
                        )

        # ---- stage 2: attention ----
        with tc.tile_pool(name="expp", bufs=2) as expp, tc.tile_pool(
            name="oscp", bufs=8
        ) as oscp, tc.tile_pool(name="rlp", bufs=2) as rlp, tc.tile_pool(
            name="gxp", bufs=3
        ) as gxp, tc.tile_pool(name="otp", bufs=3) as otp, tc.tile_pool(
            name="ps_s", bufs=3, space="PSUM"
        ) as ps_s, tc.tile_pool(name="ps_o", bufs=2, space="PSUM") as ps_o, tc.tile_pool(
            name="ps_l", bufs=1, space="PSUM"
        ) as ps_l, tc.tile_pool(name="ps_t", bufs=2, space="PSUM") as ps_t:
            for b in range(NB):
                expst = expp.tile([128, NJ, IB], bf16, tag="expst")
                for j in range(NJ):
                    ps = ps_s.tile([128, IB], f32, tag="s")
                    nc.tensor.matmul(
                        ps,
                        k_sb[:, j * 128 : (j + 1) * 128],
                        q_sb[:, b * IB : (b + 1) * IB],
                        start=True,
                        stop=True,
                    )
                    nc.scalar.activation(
                        out=expst[:, j, :], in_=ps, func=Exp, bias=0.0, scale=1.0
                    )
                # l[i] = sum_j exp(S^T[j, i]) via ones-stationary matmuls
                pl = ps_l.tile([1, IB], f32, tag="l")
                for j in range(NJ):
                    nc.tensor.matmul(
                        pl,
                        onesc_sb,
                        expst[:, j, :],
                        start=(j == 0),
                        stop=(j == NJ - 1),
                    )
                rl_row = rlp.tile([1, IB], f32, tag="rlrow")
                nc.vector.reciprocal(rl_row, pl)
                # bounce through DRAM to redistribute [1, IB] -> [128, NS]
                nc.sync.dma_start(out=lscratch[b : b + 1, :], in_=rl_row)
                rl_col = rlp.tile([128, NS], f32, tag="rlcol")
                nc.sync.dma_start(
                    out=rl_col, in_=lscratch[b, :].rearrange("(s p) -> p s", p=128)
                )
                osc_tiles = []
                for s in range(NS):
                    po = ps_o.tile([128, C], f32, tag="o")
                    for j in range(NJ):
                        nc.tensor.matmul(
                            po,
                            expst[:, j, s * 128 : (s + 1) * 128],
                            vT_sb[:, j, :],
                            start=(j == 0),
                            stop=(j == NJ - 1),
                        )
                    osc = oscp.tile([128, C], bf16, tag="osc")
                    nc.scalar.activation(
                        out=osc,
                        in_=po,
                        func=Copy,
                        bias=0.0,
                        scale=rl_col[:, s : s + 1],
                    )
                    osc_tiles.append(osc)
                for cc in range(4):
                    pt = ps_t.tile([128, IB], bf16, tag="t")
                    for s in range(NS):
                        nc.tensor.transpose(
                            pt[:, s * 128 : (s + 1) * 128],
                            osc_tiles[s][:, cc * 128 : (cc + 1) * 128],
                            eye_sb,
                        )
                    g = gxp.tile([128, IB], f32, tag="g")
                    nc.sync.dma_start(
                        out=g,
                        in_=gamma[cc * 128 : (cc + 1) * 128, b * IB : (b + 1) * IB],
                    )
                    xx = gxp.tile([128, IB], f32, tag="xx")
                    nc.sync.dma_start(
                        out=xx,
                        in_=x[cc * 128 : (cc + 1) * 128, b * IB : (b + 1) * IB],
                    )
                    ot = otp.tile([128, IB], f32, tag="ot")
                    nc.vector.tensor_mul(ot, pt, g)
                    nc.vector.tensor_add(ot, ot, xx)
                    nc.sync.dma_start(
                        out=out[cc * 128 : (cc + 1) * 128, b * IB : (b + 1) * IB],
                        in_=ot,
                    )

    with tile.TileContext(nc) as tc:
        with ExitStack() as ctx:
            if loop_reps is None:
                body(tc, ctx)
            else:
                with tc.For_i(0, loop_reps, 1):
                    body(tc, ctx)
    nc.compile()
    return nc


def _prep_host_inputs(inputs):
    """Fold BN scales into weights, transpose, build per-core input maps."""
    import ml_dtypes

    f = lambda a: np.ascontiguousarray(np.asarray(a, dtype=np.float32))
    x = f(inputs["x"]).reshape(B, C, HW)
    wqT = f((np.asarray(inputs["sq"])[:, None] * np.asarray(inputs["Wq"])).T)
    wkT = f((np.asarray(inputs["sk"])[:, None] * np.asarray(inputs["Wk"])).T)
    wvT = f((np.asarray(inputs["sv"])[:, None] * np.asarray(inputs["Wv"])).T)
    shared = {
        "wqT": wqT,
        "wkT": wkT,
        "wvT": wvT,
        "bq": f(inputs["bq"]).reshape(CQK, 1),
        "bk": f(inputs["bk"]).reshape(CQK, 1),
        "bv": f(inputs["bv"]).reshape(1, C),
        "gamma": f(inputs["gamma"]).reshape(C, HW),
        "onesr": np.ones((1, 128), np.float32),
        "eye": np.eye(128, dtype=np.float32).astype(ml_dtypes.bfloat16),
    }
    return [dict(shared, x=x[i]) for i in range(NCORES)]


def kernel(**inputs):
    from concourse.bass_utils import run_bass_kernel_spmd

    if "nc" not in _STATE:
        _STATE["nc"] = build_program()
    nc = _STATE["nc"]
    in_maps = _prep_host_inputs(inputs)
    res = run_bass_kernel_spmd(nc, in_maps, list(range(NCORES)))
    out = np.stack([res.results[i]["out"] for i in range(NCORES)])
    return out.reshape(B, C, H, W).astype(np.float32)


if __name__ == "__main__":
    rng = np.random.default_rng(0)
    demo = {
        "x": rng.standard_normal((B, C, H, W), dtype=np.float32),
        "Wq": rng.standard_normal((CQK, C), dtype=np.float32) * 0.02,
        "Wk": rng.standard_normal((CQK, C), dtype=np.float32) * 0.02,
        "Wv": rng.standard_normal((C, C), dtype=np.float32) * 0.02,
        "sq": rng.uniform(0.5, 1.5, CQK).astype(np.float32),
        "bq": rng.standard_normal(CQK).astype(np.float32) * 0.1,
        "sk": rng.uniform(0.5, 1.5, CQK).astype(np.float32),
        "bk": rng.standard_normal(CQK).astype(np.float32) * 0.1,
        "sv": rng.uniform(0.5, 1.5, C).astype(np.float32),
        "bv": rng.standard_normal(C).astype(np.float32) * 0.1,
        "gamma": rng.standard_normal((C, H, W), dtype=np.float32) * 0.1,
    }
    y = kernel(**demo)
    print("kernel output:", y.shape, y.dtype, float(np.abs(y).max()))



# revision 21
# speedup vs baseline: 4.1114x; 4.1114x over previous
"""Position-attention layer (dense_transformer) for Trainium2, 8 NeuronCores.

Data-parallel over batch B=8: one batch element per core. Per core:
  q = relu(Wq' x + bq)            [80, 4096]   (BN scales folded on host)
  k = relu(Wk' x + bk)            [80, 4096]
  vT = relu(x^T Wv'^T + bv)       [4096, 512]  (computed directly transposed)
  S^T[j,i] = sum_c k[c,j] q[c,i]  (energy, j on partitions)
  P = exp(S^T - 2)                (fp8e5m2; global shift, S in [0.1, 11.6])
  l[i] = sum_j P[j,i]             (piggyback N=1 matmuls -> [128,1] psum cols)
  num[i,c] = sum_j P[j,i] vT[j,c] (fp8 DoubleRow matmuls, contraction 256/MM)
  oscg = (num * 1/l) .* gammaT    (fused DVE pass; gamma pre-transposed on host)
  out[c,i] = oscg^T[c,i] + x[c,i] (PE transpose + DVE add)

All projections run as fp8e4m3 DoubleRow matmuls with power-of-2 scales
(x*16, W*64, q/k/v*32) folded into the PSUM-eviction scale; biases are
folded into extra DoubleRow matmuls (bias*8 stationary/moving against a
constant-128 fp8 row). The energy matmul contracts 80 channels as
DoubleRow Ki=40 fp8. Scale bookkeeping cancels exactly in num/l.

The attention loop is software-pipelined: energy+exp of block b overlap
the PV/l/transpose consumption of block b-1, sized so the ACT engine's
exp stream (the secondary bottleneck) never stalls the PE.
"""

import sys

sys.path.insert(0, "/opt/trn_rl_repo")

import numpy as np

B, C, H, W = 8, 512, 64, 64
HW = H * W          # 4096
CQK = 80
NCORES = 8
IB = 512            # i-block size for the attention stage
NB = HW // IB       # 8 i-blocks
NS = IB // 128      # 4 i-subtiles per block
NJ = HW // 128      # 32 j-tiles

XS = 16.0           # x fp8 scale
WS = 64.0           # weight fp8 scale
QS = 32.0           # q/k/v fp8 scale
EXP_SHIFT = 2.0     # P = exp(S - EXP_SHIFT); S_max ~ 11.6 -> P_max ~ e^9.6 < e5m2 max
TRICK_STEADY = frozenset({1, 4, 6, 9, 11, 14})   # fast-exp groups, steady blocks
TRICK_B0 = frozenset({1, 3, 5, 7, 9, 11, 13, 15})  # fast-exp groups, block 0

_STATE = {}


def build_program(loop_reps=None):
    """Build the per-core Bass program. If loop_reps is set, wrap the whole
    kernel body in a hardware For_i loop (used for timing benchmarks only)."""
    from contextlib import ExitStack

    import concourse.bass as bass  # noqa: F401
    import concourse.tile as tile
    from concourse import bacc, mybir

    f32 = mybir.dt.float32
    i8 = mybir.dt.int8
    bf16 = mybir.dt.bfloat16
    f8e4 = mybir.dt.float8e4
    f8e5 = mybir.dt.float8e5
    Relu = mybir.ActivationFunctionType.Relu
    Exp = mybir.ActivationFunctionType.Exp
    DR = mybir.MatmulPerfMode.DoubleRow
    mult = mybir.AluOpType.mult
    amax = mybir.AluOpType.max

    nc = bacc.Bacc("TRN2", target_bir_lowering=False, debug=False)
    x = nc.declare_dram_parameter("x", [C, HW], f32, isOutput=False)
    x8 = nc.declare_dram_parameter("x8", [128, 4, HW], f8e4, isOutput=False)
    wq8 = nc.declare_dram_parameter("wq8", [128, 4, CQK], f8e4, isOutput=False)
    wk8 = nc.declare_dram_parameter("wk8", [128, 4, CQK], f8e4, isOutput=False)
    wv8 = nc.declare_dram_parameter("wv8", [128, 4, C], f8e4, isOutput=False)
    bq4 = nc.declare_dram_parameter("bq4", [1, 2, CQK], f8e4, isOutput=False)
    bk4 = nc.declare_dram_parameter("bk4", [1, 2, CQK], f8e4, isOutput=False)
    bv4 = nc.declare_dram_parameter("bv4", [1, 2, C], f8e4, isOutput=False)
    ones_row = nc.declare_dram_parameter("ones_row", [1, 2, C], f8e4, isOutput=False)
    ones_col = nc.declare_dram_parameter("ones_col", [1, 2, 128], f8e4, isOutput=False)
    gammaT = nc.declare_dram_parameter("gammaT", [HW, C], bf16, isOutput=False)
    eye = nc.declare_dram_parameter("eye", [128, 128], bf16, isOutput=False)
    out = nc.declare_dram_parameter("out", [C, HW], f32, isOutput=True)

    def body(tc, ctx):
        persist = ctx.enter_context(tc.tile_pool(name="persist", bufs=1))
        x8_sb = persist.tile([128, 4, HW], f8e4, tag="x8")
        wq8_sb = persist.tile([128, 4, CQK], f8e4, tag="wq8")
        wk8_sb = persist.tile([128, 4, CQK], f8e4, tag="wk8")
        wv8_sb = persist.tile([128, 4, C], f8e4, tag="wv8")
        bq4_sb = persist.tile([1, 2, CQK], f8e4, tag="bq4")
        bk4_sb = persist.tile([1, 2, CQK], f8e4, tag="bk4")
        bv4_sb = persist.tile([1, 2, C], f8e4, tag="bv4")
        onesr_sb = persist.tile([1, 2, C], f8e4, tag="onesr")
        onesc_sb = persist.tile([1, 2, 128], f8e4, tag="onesc")
        ones32_sb = persist.tile([128, 2, 16], f8e4, tag="ones32")
        nshift_sb = persist.tile([128, 1], f32, tag="nshift")
        eye_sb = persist.tile([128, 128], bf16, tag="eye")
        q8_sb = persist.tile([64, 2, HW], f8e4, tag="q8")
        k8_sb = persist.tile([64, 2, HW], f8e4, tag="k8")
        qhi_sb = persist.tile([CQK, HW], f8e4, tag="qhi")
        khi_sb = persist.tile([CQK, HW], f8e4, tag="khi")
        vT8_sb = persist.tile([128, NJ, C], f8e4, tag="vT8")

        nc.sync.dma_start(out=wk8_sb, in_=wk8[:, :, :])
        nc.sync.dma_start(out=wq8_sb, in_=wq8[:, :, :])
        nc.sync.dma_start(out=wv8_sb, in_=wv8[:, :, :])
        for kc in range(4):
            nc.sync.dma_start(out=x8_sb[:, kc, :], in_=x8[:, kc, :])
        nc.sync.dma_start(out=bq4_sb, in_=bq4[:, :, :])
        nc.sync.dma_start(out=bk4_sb, in_=bk4[:, :, :])
        nc.sync.dma_start(out=bv4_sb, in_=bv4[:, :, :])
        nc.sync.dma_start(out=onesr_sb, in_=ones_row[:, :, :])
        nc.sync.dma_start(out=onesc_sb, in_=ones_col[:, :, :])
        nc.sync.dma_start(out=eye_sb, in_=eye[:, :])
        nc.vector.memset(ones32_sb, QS)
        nc.vector.memset(nshift_sb, -EXP_SHIFT)

        # ---- stage 1a: q/k projections (fp8 DoubleRow, bias via DR matmul) ----
        # psum = XS*WS*(W@x) + XS*WS*b ; evict q8 = QS*relu(psum/(XS*WS))
        ev_scale = QS / (XS * WS)
        with tc.tile_pool(name="pqpool", bufs=3, space="PSUM") as pqpool:
            # q8/k8 pairing: slot t=0 holds channels 0:64, slot t=1 holds
            # channels 64:80 on partitions 0:16 (rest zero-padded)
            nc.gpsimd.memset(q8_sb[:, 1, :], 0)
            nc.gpsimd.memset(k8_sb[:, 1, :], 0)
            for w8_sb, b4_sb, dst8, hi8 in (
                (wk8_sb, bk4_sb, k8_sb, khi_sb),
                (wq8_sb, bq4_sb, q8_sb, qhi_sb),
            ):
                for ns in range(4):  # 1024-wide n-supertiles
                    pq = pqpool.tile([CQK, 2, 512], f32, tag="pq")
                    for h in range(2):
                        n0 = ns * 1024 + h * 512
                        for cp in range(2):
                            nc.tensor.matmul(
                                pq[:, h, :],
                                w8_sb[:, 2 * cp : 2 * cp + 2, :],
                                x8_sb[:, 2 * cp : 2 * cp + 2, n0 : n0 + 512],
                                start=(cp == 0),
                                stop=False,
                                perf_mode=DR,
                            )
                        nc.tensor.matmul(
                            pq[:, h, :],
                            b4_sb,
                            onesr_sb,
                            start=False,
                            stop=True,
                            perf_mode=DR,
                        )
                    # evict rows 0:64 on ACT (relu+scale), rows 64:80 on DVE
                    nc.scalar.activation(
                        out=dst8[:, 0, ns * 1024 : (ns + 1) * 1024].rearrange(
                            "p (h n) -> p h n", h=2
                        ),
                        in_=pq[0:64, :, :],
                        func=Relu,
                        bias=0.0,
                        scale=ev_scale,
                    )
                    nc.vector.tensor_scalar(
                        out=hi8[64:80, ns * 1024 : (ns + 1) * 1024].rearrange(
                            "p (h n) -> p h n", h=2
                        ),
                        in0=pq[64:80, :, :],
                        scalar1=ev_scale,
                        scalar2=0.0,
                        op0=mult,
                        op1=amax,
                    )
                nc.sync.dma_start(out=dst8[0:16, 1, :], in_=hi8[64:80, :])

        # ---- stage 1b: v projection transposed (fp8 DoubleRow + DR bias),
        # evictions alternate ACT/DVE; emitted as a callback per 2-j group so
        # the attention stage can interleave block-0 energy with it.
        def emit_v_group(pvpool, vg):
            pv = pvpool.tile([128, 2, 512], f32, tag="pv", name=f"pv{vg}")
            for jj in range(2):
                j = 2 * vg + jj
                for cp in range(2):
                    nc.tensor.matmul(
                        pv[:, jj, :],
                        x8_sb[:, 2 * cp : 2 * cp + 2, j * 128 : (j + 1) * 128],
                        wv8_sb[:, 2 * cp : 2 * cp + 2, :],
                        start=(cp == 0),
                        stop=False,
                        perf_mode=DR,
                    )
                nc.tensor.matmul(
                    pv[:, jj, :],
                    onesc_sb,
                    bv4_sb,
                    start=False,
                    stop=True,
                    perf_mode=DR,
                )
            if vg % 2 == 0:
                nc.scalar.activation(
                    out=vT8_sb[:, 2 * vg : 2 * vg + 2, :],
                    in_=pv,
                    func=Relu,
                    bias=0.0,
                    scale=ev_scale,
                )
            else:
                nc.vector.tensor_scalar(
                    out=vT8_sb[:, 2 * vg : 2 * vg + 2, :],
                    in0=pv,
                    scalar1=ev_scale,
                    scalar2=0.0,
                    op0=mult,
                    op1=amax,
                )

        # ---- stage 2: software-pipelined attention ----
        exp_scale = 1.0 / (QS * QS)
        # Schraudolph fast-exp emitting fp8e5m2 bit patterns directly:
        # e5m2 bits for 2^y are int8(4*y + 4*15) (S>=0 so no sign risk);
        # y = (S/1024 - EXP_SHIFT)*log2(e); the -0.5-ish magic balances the
        # piecewise-linear interpolation error like the classic f32 trick.
        log2e = 1.4426950408889634
        f_a = 4.0 * log2e / (QS * QS)
        f_b = 4.0 * (15.0 - EXP_SHIFT * log2e) - 0.237
        with tc.tile_pool(name="expp", bufs=2) as expp, tc.tile_pool(
            name="t32p", bufs=3
        ) as t32p, tc.tile_pool(
            name="oscp", bufs=6
        ) as oscp, tc.tile_pool(name="rlp", bufs=2) as rlp, tc.tile_pool(
            name="gxp", bufs=4
        ) as gxp, tc.tile_pool(name="otp", bufs=3) as otp, tc.tile_pool(
            name="ps_s", bufs=2, space="PSUM"
        ) as ps_s:

            # pvpool (4 banks) lives only through block 0's energy phase; the
            # consume pools (ps_o/ps_l/ps_t, 4 banks) open after it closes.
            pv_ctx = ExitStack()
            pvpool = pv_ctx.enter_context(
                tc.tile_pool(name="pvpool", bufs=2, space="PSUM")
            )
            late_ctx = ExitStack()
            P = {}
            state = {}

            def emit_energy_group(b, g, trick):
                ps = ps_s.tile([128, 2, 512], f32, tag="s", name=f"s{b}_{g}")
                for jj in range(2):
                    j = 2 * g + jj
                    nc.tensor.matmul(
                        ps[:, jj, :],
                        k8_sb[:, :, j * 128 : (j + 1) * 128],
                        q8_sb[:, :, b * IB : (b + 1) * IB],
                        start=True,
                        stop=True,
                        perf_mode=DR,
                    )
                if trick:
                    # single DVE pass: int8 convert writes e5m2 bit patterns
                    nc.vector.tensor_scalar(
                        out=state[("exp", b)][:, 2 * g : 2 * g + 2, :].bitcast(i8),
                        in0=ps,
                        scalar1=f_a,
                        scalar2=f_b,
                        op0=mult,
                        op1=mybir.AluOpType.add,
                    )
                else:
                    nc.scalar.activation(
                        out=state[("exp", b)][:, 2 * g : 2 * g + 2, :],
                        in_=ps,
                        func=Exp,
                        bias=nshift_sb,
                        scale=exp_scale,
                    )

            def emit_consume_group(b, g):
                expst = state[("exp", b)]
                if g == 0:
                    state[("pl", b)] = P["ps_l"].tile(
                        [128, 4], f32, tag="pl", name=f"pl{b}"
                    )
                for m in range(4 * g, 4 * g + 4):
                    s, jj = divmod(m, 16)
                    if jj == 0:
                        state[("po", b, s)] = P["ps_o"].tile(
                            [128, 512], f32, tag="po", name=f"po{b}_{s}"
                        )
                    po = state[("po", b, s)]
                    nc.tensor.matmul(
                        po,
                        expst[:, 2 * jj : 2 * jj + 2, s * 128 : (s + 1) * 128],
                        vT8_sb[:, 2 * jj : 2 * jj + 2, :],
                        start=(jj == 0),
                        stop=(jj == 15),
                        perf_mode=DR,
                        skip_group_check=True,
                    )
                    nc.tensor.matmul(
                        state[("pl", b)][:, s : s + 1],
                        expst[:, 2 * jj : 2 * jj + 2, s * 128 : (s + 1) * 128],
                        ones32_sb[:, :, 0:1],
                        start=(jj == 0),
                        stop=(jj == 15),
                        perf_mode=DR,
                        skip_group_check=True,
                    )
                    if jj == 15:
                        rl4 = state[("rl", b)]
                        nc.vector.reciprocal(
                            rl4[:, s : s + 1], state[("pl", b)][:, s : s + 1]
                        )
                        gt = gxp.tile([128, C], bf16, tag="gt", name=f"gt{b}_{s}")
                        nc.sync.dma_start(
                            out=gt,
                            in_=gammaT[b * IB + s * 128 : b * IB + (s + 1) * 128, :],
                        )
                        oscg = oscp.tile([128, C], bf16, tag="oscg", name=f"og{b}_{s}")
                        nc.vector.scalar_tensor_tensor(
                            out=oscg,
                            in0=po,
                            scalar=rl4[:, s : s + 1],
                            in1=gt,
                            op0=mult,
                            op1=mult,
                        )
                        state[("oscg", b, s)] = oscg

            def emit_block_tail(b):
                pt = P["ps_t"].tile([128, 2, 512], bf16, tag="pt", name=f"pt{b}")
                for cc in range(4):
                    h = cc % 2
                    for s in range(4):
                        nc.tensor.transpose(
                            pt[:, h, s * 128 : (s + 1) * 128],
                            state[("oscg", b, s)][:, cc * 128 : (cc + 1) * 128],
                            eye_sb,
                        )
                    xx = gxp.tile([128, IB], f32, tag="xx", name=f"xx{b}_{cc}")
                    nc.sync.dma_start(
                        out=xx,
                        in_=x[cc * 128 : (cc + 1) * 128, b * IB : (b + 1) * IB],
                    )
                    ot = otp.tile([128, IB], f32, tag="ot", name=f"ot{b}_{cc}")
                    nc.vector.tensor_add(ot, pt[:, h, :], xx)
                    nc.sync.dma_start(
                        out=out[cc * 128 : (cc + 1) * 128, b * IB : (b + 1) * IB],
                        in_=ot,
                    )

            for b in range(NB + 1):
                if b == 1:
                    pv_ctx.close()
                    P["ps_o"] = late_ctx.enter_context(
                        tc.tile_pool(name="ps_o", bufs=2, space="PSUM")
                    )
                    P["ps_l"] = late_ctx.enter_context(
                        tc.tile_pool(name="ps_l", bufs=1, space="PSUM")
                    )
                    P["ps_t"] = late_ctx.enter_context(
                        tc.tile_pool(name="ps_t", bufs=1, space="PSUM")
                    )
                if b < NB:
                    state[("exp", b)] = expp.tile(
                        [128, NJ, IB], f8e5, tag="expst", name=f"expst{b}"
                    )
                    state[("rl", b)] = rlp.tile([128, 4], f32, tag="rl", name=f"rl{b}")
                trick_set = TRICK_B0 if b == 0 else TRICK_STEADY
                for g in range(16):
                    if b == 0:
                        # interleave the v projection with block-0 energy
                        emit_v_group(pvpool, g)
                    if b < NB:
                        emit_energy_group(b, g, g in trick_set)
                    if b > 0:
                        emit_consume_group(b - 1, g)
                if b > 0:
                    emit_block_tail(b - 1)
            late_ctx.close()

    with tile.TileContext(nc) as tc:
        with ExitStack() as ctx:
            if loop_reps is None:
                body(tc, ctx)
            else:
                with tc.For_i(0, loop_reps, 1):
                    body(tc, ctx)
    nc.compile()
    return nc


def _prep_host_inputs(inputs):
    """Fold BN scales into weights, quantize to fp8, build per-core inputs."""
    import ml_dtypes

    e4 = ml_dtypes.float8_e4m3
    f = lambda a: np.ascontiguousarray(np.asarray(a, dtype=np.float32))

    x = f(inputs["x"]).reshape(B, C, HW)
    # [C, n] -> [128, 4, n] chunked layout (chunk kc holds channels kc*128+p)
    chunk = lambda a: np.ascontiguousarray(
        a.reshape(4, 128, -1).transpose(1, 0, 2)
    )
    # [B, C, HW] -> [B, 128, 4, HW]
    x8 = np.ascontiguousarray(
        (x * XS).astype(e4).reshape(B, 4, 128, HW).transpose(0, 2, 1, 3)
    )

    wqT = f((np.asarray(inputs["sq"])[:, None] * np.asarray(inputs["Wq"])).T)
    wkT = f((np.asarray(inputs["sk"])[:, None] * np.asarray(inputs["Wk"])).T)
    wvT = f((np.asarray(inputs["sv"])[:, None] * np.asarray(inputs["Wv"])).T)

    pair0 = lambda v: np.stack([v, np.zeros_like(v)], axis=0)[None]  # [1,2,n]

    shared = {
        "wq8": chunk((wqT * WS).astype(e4)),
        "wk8": chunk((wkT * WS).astype(e4)),
        "wv8": chunk((wvT * WS).astype(e4)),
        "bq4": pair0((8.0 * f(inputs["bq"])).astype(e4)),
        "bk4": pair0((8.0 * f(inputs["bk"])).astype(e4)),
        "bv4": pair0((8.0 * f(inputs["bv"])).astype(e4)),
        "ones_row": pair0(np.full(C, 128.0, np.float32).astype(e4)),
        "ones_col": pair0(np.full(128, 128.0, np.float32).astype(e4)),
        "gammaT": np.ascontiguousarray(
            f(inputs["gamma"]).reshape(C, HW).T.astype(ml_dtypes.bfloat16)
        ),
        "eye": np.eye(128, dtype=np.float32).astype(ml_dtypes.bfloat16),
    }
    return [
        dict(shared, x=np.ascontiguousarray(x[i]), x8=x8[i])
        for i in range(NCORES)
    ]


def kernel(**inputs):
    from concourse.bass_utils import run_bass_kernel_spmd

    if "nc" not in _STATE:
        _STATE["nc"] = build_program()
    nc = _STATE["nc"]
    in_maps = _prep_host_inputs(inputs)
    res = run_bass_kernel_spmd(nc, in_maps, list(range(NCORES)))
    out = np.stack([res.results[i]["out"] for i in range(NCORES)])
    return out.reshape(B, C, H, W).astype(np.float32)


if __name__ == "__main__":
    rng = np.random.default_rng(0)
    demo = {
        "x": rng.standard_normal((B, C, H, W), dtype=np.float32),
        "Wq": rng.standard_normal((CQK, C), dtype=np.float32) * 0.02,
        "Wk": rng.standard_normal((CQK, C), dtype=np.float32) * 0.02,
        "Wv": rng.standard_normal((C, C), dtype=np.float32) * 0.02,
        "sq": rng.uniform(0.5, 1.5, CQK).astype(np.float32),
        "bq": rng.standard_normal(CQK).astype(np.float32) * 0.1,
        "sk": rng.uniform(0.5, 1.5, CQK).astype(np.float32),
        "bk": rng.standard_normal(CQK).astype(np.float32) * 0.1,
        "sv": rng.uniform(0.5, 1.5, C).astype(np.float32),
        "bv": rng.standard_normal(C).astype(np.float32) * 0.1,
        "gamma": rng.standard_normal((C, H, W), dtype=np.float32) * 0.1,
    }
    y = kernel(**demo)
    print("kernel output:", y.shape, y.dtype, float(np.abs(y).max()))
